# revision 1
# baseline (speedup 1.0000x reference)
"""Trainium2 Bass kernel for nn_ConvShiftLayer.

Computes, per batch element n:
    h = x[n] @ W_dense + b_dense                      (2048, 2048)
    y[t, o] = sum_{d=0..7} h[t-d, (o+d) % 2048]       (h[<0] = 0)
    a = tanh(y),  z = (y > 0) as f32   (== (tanh(y) > 0))
Returns (y, a, z) each of shape (8, 2048, 2048) f32.

Strategy: data-parallel over batch, 1 element per NeuronCore (8 cores).
Per core:
  - x is PE-transposed to xT (D on partitions).
  - h = xT.T @ W via fp32r matmuls (full-rate) into PSUM, copied to SBUF
    h tiles of 128 *overlapping* time rows (stride 121), extended by 7
    wrap columns (channel circularity).
  - The 8-tap shifted sum runs on the TensorEngine: 8 shift-matrix
    matmuls (one per tap; time shift via 0/1 stationary matrix, channel
    shift via rhs free-dim offset) + 1 bias matmul, accumulated in PSUM.
  - y -> SBUF (DVE), a = tanh on ScalarE, z = is_gt on DVE, DMA out.
"""

import sys

if "/opt/trn_rl_repo" not in sys.path:
    sys.path.insert(0, "/opt/trn_rl_repo")

import numpy as np

B, L, DIN, F = 8, 2048, 1024, 2048
WC = 8            # conv taps
PAD = WC - 1      # 7
TS = 128 - PAD    # 121 output rows per time tile
NT = (L + TS - 1) // TS   # 17 time tiles
NCH = 4           # channel chunks of 512
CW = 512          # chunk width
NCORES = 8

# consts tensor column layout (one [128, 2432] f32 input)
#   [0:1024)     U_s up-shift matrices   (s=0..7; U[k, s*128+m] = k==m+s)
#   [1024:2048)  T_d down-shift matrices (d=0..7; T[k, d*128+m] = k==m-d)
#   [2048:2176)  identity (PE transpose)
#   [2176:2304)  ones8 block  (rows 0..7 = 1)
#   [2304:2432)  L0 triangular (rows d: 1 for cols >= d)
CONST_COLS = 2432

_CACHE = {}


def _build_consts():
    c = np.zeros((128, CONST_COLS), np.float32)
    for s in range(WC):
        for m in range(128):
            if m + s < 128:
                c[m + s, s * 128 + m] = 1.0
    for d in range(WC):
        for m in range(128):
            if m - d >= 0:
                c[m - d, 1024 + d * 128 + m] = 1.0
    c[:, 2048:2176] = np.eye(128, dtype=np.float32)
    c[0:WC, 2176:2304] = 1.0
    for d in range(WC):
        c[d, 2304 + d : 2432] = 1.0
    return c


def _split_matmul_waits(nc):
    """This walrus build accepts only one sync-wait command per instruction;
    hoist extra waits onto preceding same-engine no-ops (one wait each)."""
    import concourse.mybir as mybir

    for fn in nc.m.functions:
        for blk in fn.blocks:
            newl = []
            for inst in blk.instructions:
                si = getattr(inst, "sync_info", None)
                if (
                    si is not None
                    and len(si.on_wait) > 1
                    and not isinstance(inst, mybir.InstNoOp)
                    and getattr(inst, "engine", None) is not None
                ):
                    waits = list(si.on_wait)
                    for wi, w in enumerate(waits[:-1]):
                        pre = mybir.InstNoOp(
                            name=f"{inst.name}_wsplit{wi}",
                            sync_info=mybir.SyncInfo(on_wait=[w], on_update=[]),
                            bass_nofuse=True,
                            engine=inst.engine,
                        )
                        newl.append(pre)
                    si.on_wait = waits[-1:]
                newl.append(inst)
            blk.instructions = newl


def _build_nc(mm_dtype_name="float32r", split_waits=True):
    import concourse.bass as bass
    import concourse.mybir as mybir
    from concourse import tile

    f32 = mybir.dt.float32
    mmdt = getattr(mybir.dt, mm_dtype_name)

    nc = bass.Bass("TRN2", target_bir_lowering=False, debug=False)

    x_d = nc.declare_dram_parameter("x", [L, DIN], f32, isOutput=False)
    w_d = nc.declare_dram_parameter("w", [DIN, F], f32, isOutput=False)
    bsh_d = nc.declare_dram_parameter("bsh", [128, F], f32, isOutput=False)
    cst_d = nc.declare_dram_parameter("cst", [128, CONST_COLS], f32, isOutput=False)
    y_d = nc.declare_dram_parameter("y", [L, F], f32, isOutput=True)
    a_d = nc.declare_dram_parameter("a", [L, F], f32, isOutput=True)
    z_d = nc.declare_dram_parameter("z", [L, F], f32, isOutput=True)

    KD = DIN // 128  # 8 K-tiles

    def mc(ap):
        return ap

    with tile.TileContext(nc) as tc:
        with (
            tc.tile_pool(name="wpool", bufs=1) as wpool,
            tc.tile_pool(name="xtpool", bufs=1) as xtpool,
            tc.tile_pool(name="cpool", bufs=1) as cpool,
        ):
            cst = cpool.tile([128, CONST_COLS], mmdt, tag="cst")
            nc.sync.dma_start(cst[:], cst_d[:].bitcast(mmdt))
            bsh = cpool.tile([128, F], mmdt, tag="bsh")
            nc.sync.dma_start(bsh[:], bsh_d[:].bitcast(mmdt))

            wt = []
            for k in range(KD):
                t = wpool.tile([128, F], mmdt, tag=f"w{k}")
                nc.sync.dma_start(t[:], w_d[k * 128 : (k + 1) * 128, :].bitcast(mmdt))
                wt.append(t)

            xt = [xtpool.tile([128, L], mmdt, tag=f"xt{k}", name=f"xt{k}") for k in range(KD)]

            ident = cst[:, 2048:2176].bitcast(f32)

            # ---- phase 0: transpose x (L,DIN) -> xT tiles [128 D, L t] ----
            with (
                tc.tile_pool(name="xstage", bufs=2) as xstage,
                tc.tile_pool(name="psum0", bufs=4, space="PSUM") as psum0,
            ):
                for i in range(L // 128):
                    xs = xstage.tile([128, DIN], f32, tag="xs")
                    nc.sync.dma_start(xs[:], x_d[i * 128 : (i + 1) * 128, :])
                    for k in range(KD):
                        tp = psum0.tile([128, 128], f32, tag="tp")
                        nc.tensor.transpose(
                            tp[:], xs[:, k * 128 : (k + 1) * 128], ident
                        )
                        nc.vector.tensor_copy(
                            xt[k][:, i * 128 : (i + 1) * 128], tp[:]
                        )

            # ---- phase 1: per time tile ----
            with (
                tc.tile_pool(name="hpool", bufs=2) as hpool,
                tc.tile_pool(name="opool", bufs=4) as opool,
                tc.tile_pool(name="psum_h", bufs=4, space="PSUM") as psum_h,
                tc.tile_pool(name="psum_y", bufs=4, space="PSUM") as psum_y,
            ):
                for i in range(NT):
                    t0 = TS * i
                    My = min(TS, L - t0)
                    if i == 0:
                        hlo, Mh = 0, 128
                    else:
                        hlo = t0 - PAD
                        Mh = min(L, t0 + TS) - hlo

                    hs = hpool.tile([128, F + PAD], mmdt, tag="hs")

                    # h = xT.T @ W  (+ wrap columns)
                    for n in range(NCH):
                        hp = psum_h.tile([128, CW], f32, tag="hp")
                        for k in range(KD):
                            nc.tensor.matmul(
                                hp[0:Mh, :],
                                mc(xt[k][:, hlo : hlo + Mh]),
                                mc(wt[k][:, n * CW : (n + 1) * CW]),
                                start=(k == 0),
                                stop=(k == KD - 1),
                            )
                        nc.vector.tensor_copy(
                            hs[0:Mh, n * CW : (n + 1) * CW], hp[0:Mh, :]
                        )
                    nc.vector.tensor_copy(
                        hs[0:Mh, F : F + PAD], hs[0:Mh, 0:PAD]
                    )

                    # y = sum_d shift_d(h) + bias  (all on PE, PSUM-accumulated)
                    for n in range(NCH):
                        yp = psum_y.tile([128, CW], f32, tag="yp")
                        ys = opool.tile([128, CW], f32, tag="ys", name="ys")
                        as_ = opool.tile([128, CW], f32, tag="as", name="as_")
                        zs = opool.tile([128, CW], f32, tag="zs", name="zs")
                        if i == 0:
                            blhs = cst[:, 2304 : 2304 + My]
                        else:
                            blhs = cst[:, 2176 : 2176 + My]
                        nc.tensor.matmul(
                            yp[0:My, :],
                            mc(blhs),
                            mc(bsh[:, n * CW : (n + 1) * CW]),
                            start=True,
                            stop=False,
                        )
                        for j in range(WC):
                            if i == 0:
                                lhs = cst[0:Mh, 1024 + j * 128 : 1024 + j * 128 + My]
                                off = j
                            else:
                                lhs = cst[0:Mh, j * 128 : j * 128 + My]
                                off = PAD - j
                            nc.tensor.matmul(
                                yp[0:My, :],
                                mc(lhs),
                                mc(hs[0:Mh, n * CW + off : n * CW + off + CW]),
                                start=False,
                                stop=(j == WC - 1),
                            )
                        nc.vector.tensor_copy(ys[0:My, :], yp[0:My, :])
                        nc.scalar.activation(
                            as_[0:My, :],
                            ys[0:My, :],
                            mybir.ActivationFunctionType.Tanh,
                        )
                        nc.vector.tensor_scalar(
                            zs[0:My, :],
                            ys[0:My, :],
                            0.0,
                            None,
                            mybir.AluOpType.is_gt,
                        )
                        csl = slice(n * CW, (n + 1) * CW)
                        nc.sync.dma_start(y_d[t0 : t0 + My, csl], ys[0:My, :])
                        nc.sync.dma_start(a_d[t0 : t0 + My, csl], as_[0:My, :])
                        nc.sync.dma_start(z_d[t0 : t0 + My, csl], zs[0:My, :])

    if split_waits:
        _split_matmul_waits(nc)
    return nc


def _get_nc():
    key = "nc"
    if key not in _CACHE:
        _CACHE[key] = _build_nc()
    return _CACHE[key]


def kernel(x, W_dense, b_dense):
    from concourse.bass_utils import run_bass_kernel_spmd

    x = np.asarray(x, np.float32)
    W = np.ascontiguousarray(np.asarray(W_dense, np.float32))
    b = np.asarray(b_dense, np.float32)

    bext = np.concatenate([b, b[:PAD]])
    bsh = np.zeros((128, F), np.float32)
    for d in range(WC):
        bsh[d] = bext[d : d + F]
    cst = _build_consts()

    nc = _get_nc()
    in_maps = [
        {"x": np.ascontiguousarray(x[n]), "w": W, "bsh": bsh, "cst": cst}
        for n in range(NCORES)
    ]
    res = run_bass_kernel_spmd(nc, in_maps, list(range(NCORES))).results

    y = np.stack([res[n]["y"] for n in range(NCORES)])
    a = np.stack([res[n]["a"] for n in range(NCORES)])
    z = np.stack([res[n]["z"] for n in range(NCORES)])
    return y, a, z

